# revision 17
# baseline (speedup 1.0000x reference)
"""Trainium2 Bass kernel for BasicPGCBlock (promoted v8):
baseline structure with S5 identity-matmuls emitted ahead of the previous
slab's conv groups (PE queue order matches data readiness), S5 last in the
accumulation chain, and first/last slab chains split into 8-row halves so
conv chunks release earlier at startup and the solo-conv tail shrinks.
Sharding: data-parallel over batch, 1 image per NeuronCore (8 cores).
Models 231µs vs 246µs baseline; paired-HW best in low-noise conditions."""
import sys
sys.path.insert(0, "/opt/trn_rl_repo")
import numpy as np
import ml_dtypes
BF16 = ml_dtypes.bfloat16
B, C, H, W = 8, 256, 96, 96
HP, WP = H + 4, W + 4
SLABS = ((0, 8), (8, 16), (24, 16), (40, 16), (56, 16), (72, 16), (88, 8))
CHUNK = 4
OFFS = (-2, 0, 2)
MS = (0, 1, 2, 4, 5, 8)
_cache = {}

def _build(repeats=1, loop=None):
    import concourse.mybir as mybir
    from concourse import bacc
    from concourse.tile import TileContext
    dt = mybir.dt
    nc = bacc.Bacc("TRN2", target_bir_lowering=False, debug=False)
    xp = nc.dram_tensor("xp", (128, 2, HP, WP), dt.bfloat16, kind="ExternalInput").ap()
    cpl = nc.dram_tensor("cpl", (128, 6, H, W), dt.bfloat16, kind="ExternalInput").ap()
    wts = nc.dram_tensor("wts", (2, 128, 9 * 2 * 128), dt.bfloat16, kind="ExternalInput").ap()
    bias = nc.dram_tensor("bias", (128, 2), dt.float32, kind="ExternalInput").ap()
    ident = nc.dram_tensor("ident", (128, 128), dt.bfloat16, kind="ExternalInput").ap()
    y = nc.dram_tensor("y", (2, 128, H, W), dt.float32, kind="ExternalOutput").ap()
    with TileContext(nc) as tc:
        with (
            tc.tile_pool(name="const", bufs=1) as constp,
            tc.tile_pool(name="smpool", bufs=1) as smpool,
            tc.tile_pool(name="io", bufs=2) as iop,
            tc.tile_pool(name="tmp", bufs=1) as tmp,
            tc.tile_pool(name="outp", bufs=6) as outp,
            tc.tile_pool(name="psum", bufs=8, space="PSUM") as psp,
        ):
            id_sb = constp.tile([128, 128], dt.bfloat16)
            nc.sync.dma_start(out=id_sb, in_=ident)
            w_sb = constp.tile([128, 2, 9 * 2 * 128], dt.bfloat16)
            b_sb = constp.tile([128, 2], dt.float32)
            def load_consts():
                nc.sync.dma_start(out=w_sb[:, 0], in_=wts[0])
                nc.sync.dma_start(out=w_sb[:, 1], in_=wts[1])
                nc.sync.dma_start(out=b_sb, in_=bias)
            sm = smpool.tile([128, 2, HP, WP], dt.bfloat16)
            nc.vector.memset(sm[:, :, 0:2, :], 0.0)
            nc.vector.memset(sm[:, :, HP - 2 : HP, :], 0.0)
            nc.vector.memset(sm[:, :, 2 : HP - 2, 0:2], 0.0)
            nc.vector.memset(sm[:, :, 2 : HP - 2, WP - 2 : WP], 0.0)
            NR = 16
            def smooth_front(r0, nr):
                xs = iop.tile([128, 2, NR + 4, WP], dt.bfloat16, name="xs")[:, :, : nr + 4, :]
                nc.sync.dma_start(out=xs, in_=xp[:, :, r0 : r0 + nr + 4, :])
                cp = iop.tile([128, 6, NR, W], dt.bfloat16, name="cp")[:, :, :nr, :]
                ch = nr // 2 if nr > 8 else nr
                nc.sync.dma_start(out=cp[:, :, :ch], in_=cpl[:, :, r0 : r0 + ch, :])
                if ch < nr:
                    nc.sync.dma_start(out=cp[:, :, ch:nr], in_=cpl[:, :, r0 + ch : r0 + nr, :])
                def cpm(m):
                    i = MS.index(m)
                    return cp[:, i : i + 1].to_broadcast([128, 2, nr, W])
                P0 = xs[:, :, :, 2 : W + 2]
                P1 = tmp.tile([128, 2, NR + 4, W], dt.bfloat16, name="P1", bufs=2)[:, :, : nr + 4]
                nc.vector.tensor_add(P1, xs[:, :, :, 1 : W + 1], xs[:, :, :, 3 : W + 3])
                P2 = tmp.tile([128, 2, NR + 4, W], dt.bfloat16, name="P2", bufs=2)[:, :, : nr + 4]
                nc.vector.tensor_add(P2, xs[:, :, :, 0:W], xs[:, :, :, 4 : W + 4])
                ctr = lambda P: P[:, :, 2 : nr + 2]
                u1 = lambda P: P[:, :, 1 : nr + 1]
                d1 = lambda P: P[:, :, 3 : nr + 3]
                u2 = lambda P: P[:, :, 0:nr]
                d2 = lambda P: P[:, :, 4 : nr + 4]
                S5 = tmp.tile([128, 2, NR, W], dt.bfloat16, name="S5", bufs=2)[:, :, :nr]
                for ct in range(2):
                    for rk in range(nr // CHUNK):
                        rs = CHUNK * rk
                        pc5 = psp.tile([128, CHUNK, W], dt.float32, name="pc5", bufs=2)
                        for j, Pv in enumerate((u2(P1), d2(P1), u1(P2), d1(P2))):
                            nc.tensor.matmul(pc5, id_sb, Pv[:, ct, rs : rs + CHUNK, :],
                                             start=(j == 0), stop=(j == 3))
                        nc.scalar.activation(S5[:, ct, rs : rs + CHUNK, :], pc5,
                                             mybir.ActivationFunctionType.Copy)
                return xs, cp, P1, P2, S5

            def smooth_rest(r0, nr, xs, cp, P1, P2, S5, flush_fn=None, eng=None):
                eng = eng if eng is not None else nc.gpsimd
                P0 = xs[:, :, :, 2 : W + 2]
                S1 = tmp.tile([128, 2, NR, W], dt.bfloat16, name="S1")[:, :, :nr]
                S2 = tmp.tile([128, 2, NR, W], dt.bfloat16, name="S2")[:, :, :nr]
                S4 = tmp.tile([128, 2, NR, W], dt.bfloat16, name="S4")[:, :, :nr]
                S8 = tmp.tile([128, 2, NR, W], dt.bfloat16, name="S8")[:, :, :nr]
                halves = ((0, nr),) if nr <= 8 else ((0, nr // 2), (nr // 2, nr // 2))
                sl = lambda: None  # namespace for full-slab shifted views
                fctr = lambda P: P[:, :, 2 : 2 + nr]
                fu1 = lambda P: P[:, :, 1 : 1 + nr]
                fd1 = lambda P: P[:, :, 3 : 3 + nr]
                nc.vector.tensor_add(S1, fu1(P0), fd1(P0))
                nc.vector.tensor_add(S1, S1, fctr(P1))
                for h0, hn in halves:
                    hs = slice(h0, h0 + hn)
                    # row-shifted views of P (offset +2 = centered) restricted to this half
                    ctr = lambda P: P[:, :, 2 + h0 : 2 + h0 + hn]
                    u1 = lambda P: P[:, :, 1 + h0 : 1 + h0 + hn]
                    d1 = lambda P: P[:, :, 3 + h0 : 3 + h0 + hn]
                    u2 = lambda P: P[:, :, h0 : h0 + hn]
                    d2 = lambda P: P[:, :, 4 + h0 : 4 + h0 + hn]
                    eng.tensor_add(S2[:, :, hs], u1(P1), d1(P1))
                    eng.tensor_add(S4[:, :, hs], u2(P0), d2(P0))
                    eng.tensor_add(S4[:, :, hs], S4[:, :, hs], ctr(P2))
                    eng.tensor_add(S8[:, :, hs], u2(P2), d2(P2))
                    def cpmh(m):
                        i = MS.index(m)
                        return cp[:, i : i + 1, h0 : h0 + hn, :].to_broadcast([128, 2, hn, W])
                    acc = tmp.tile([128, 2, NR, W], dt.bfloat16, name="acc", bufs=2)[:, :, :hn]
                    nc.vector.tensor_mul(acc, ctr(P0), cpmh(0))
                    sm_out = sm[:, :, 2 + r0 + h0 : 2 + r0 + h0 + hn, 2 : W + 2]
                    def term(S, m, last=False):
                        t = tmp.tile([128, 2, NR, W], dt.bfloat16, name="t", bufs=2)[:, :, :hn]
                        nc.vector.tensor_mul(t, S[:, :, hs], cpmh(m))
                        nc.vector.tensor_add(sm_out if last else acc, acc, t)
                    term(S2, 2)
                    term(S4, 4)
                    term(S8, 8)
                    term(S1, 1)
                    term(S5, 5, last=True)
                    if flush_fn is not None:
                        flush_fn(r0 + h0 + hn)
            def conv_group(rrs):
                for oi in range(2):
                    pcs = [psp.tile([128, CHUNK, W], dt.float32, name="pc", bufs=6) for _ in rrs]
                    for idx in range(18):
                        ki, q = idx // 9, idx % 9
                        dh, dw = OFFS[q // 3], OFFS[q % 3]
                        lhsT = w_sb[:, ki, (q * 2 + oi) * 128 : (q * 2 + oi + 1) * 128]
                        for j, rr in enumerate(rrs):
                            rhs = sm[:, ki, 2 + rr + dh : 2 + rr + CHUNK + dh, 2 + dw : 2 + dw + W]
                            nc.tensor.matmul(pcs[j], lhsT, rhs, start=(idx == 0), stop=(idx == 17))
                    for j, rr in enumerate(rrs):
                        ob = outp.tile([128, CHUNK, W], dt.float32, name="ob")
                        nc.scalar.activation(ob, pcs[j], mybir.ActivationFunctionType.Relu,
                                             bias=b_sb[:, oi : oi + 1], scale=1.0)
                        nc.sync.dma_start(out=y[oi, :, rr : rr + CHUNK, :], in_=ob)
            def body():
                pending = list(range(0, H, CHUNK))
                def flush(upto):
                    ready = [rr for rr in pending if rr + 6 <= upto or upto >= H]
                    for rr in ready:
                        pending.remove(rr)
                    if ready:
                        conv_group(ready)
                prev_end = None
                last = len(SLABS) - 1
                for si, (r0, nr) in enumerate(SLABS):
                    fr = smooth_front(r0, nr)
                    if si == 0:
                        load_consts()
                    if prev_end is not None:
                        flush(prev_end)
                    smooth_rest(r0, nr, *fr,
                                flush_fn=flush,
                                eng=nc.vector if si in (0, last) else nc.gpsimd)
                    prev_end = r0 + nr
                flush(H)
                assert not pending
            if loop is not None:
                with tc.For_i(0, loop, 1):
                    body()
            else:
                for _ in range(repeats):
                    body()
    nc.compile()
    return nc

def _prep(inputs):
    x = np.asarray(inputs["x"], np.float32)
    pm = np.asarray(inputs["perspective_map"], np.float32)
    co = np.asarray(inputs["sigma_coeffs"], np.float32)
    Wc = np.asarray(inputs["conv_w"], np.float32)
    bb = np.asarray(inputs["conv_b"], np.float32)
    p = pm[:, 0]
    sigma = np.maximum(co[0] * p**3 + co[1] * p**2 + co[2] * p + co[3], 0.5)
    t = np.exp(-1.0 / (2.0 * sigma * sigma))
    Z = 1 + 4 * t + 4 * t**2 + 4 * t**4 + 8 * t**5 + 4 * t**8
    cm = np.stack([(t**m) / Z for m in MS], axis=1).astype(BF16)
    cpl = np.ascontiguousarray(np.broadcast_to(cm[:, None], (B, 128, 6, H, W)))
    xpad = np.zeros((B, 128, 2, HP, WP), BF16)
    xpad[:, :, :, 2 : H + 2, 2 : W + 2] = (
        x.astype(BF16).reshape(B, 2, 128, H, W).transpose(0, 2, 1, 3, 4))
    Wt = Wc.transpose(1, 0, 2, 3).astype(BF16)
    wts = np.empty((2, 128, 9, 2, 128), BF16)
    for ki in range(2):
        for q in range(9):
            kh, kw = q // 3, q % 3
            for oi in range(2):
                wts[ki, :, q, oi, :] = Wt[ki * 128 : (ki + 1) * 128, oi * 128 : (oi + 1) * 128, kh, kw]
    wts = wts.reshape(2, 128, 9 * 2 * 128)
    bias_h = np.ascontiguousarray(bb.reshape(2, 128).T.astype(np.float32))
    ident = np.eye(128, dtype=BF16)
    return [{"xp": xpad[b], "cpl": cpl[b], "wts": wts, "bias": bias_h, "ident": ident} for b in range(B)]

def _get_nc(repeats=1, loop=None, **kw):
    key = ("nc", repeats, loop)
    if key not in _cache:
        _cache[key] = _build(repeats, loop)
    return _cache[key]


def run(inputs, trace=False, **kw):
    from concourse.bass_utils import run_bass_kernel_spmd

    nc = _get_nc()
    in_maps = _prep(inputs)
    res = run_bass_kernel_spmd(nc, in_maps, core_ids=list(range(B)), trace=trace, **kw)
    out = np.stack([r["y"].reshape(C, H, W) for r in res.results]).astype(np.float32)
    return out, res


def kernel(**inputs):
    out, _ = run(inputs)
    return out



# revision 18
# speedup vs baseline: 1.3252x; 1.3252x over previous
"""Trainium2 Bass kernel for BasicPGCBlock (promoted v8):
baseline structure with S5 identity-matmuls emitted ahead of the previous
slab's conv groups (PE queue order matches data readiness), S5 last in the
accumulation chain, and first/last slab chains split into 8-row halves so
conv chunks release earlier at startup and the solo-conv tail shrinks.
Sharding: data-parallel over batch, 1 image per NeuronCore (8 cores).
Models 231µs vs 246µs baseline; paired-HW best in low-noise conditions."""
import sys
sys.path.insert(0, "/opt/trn_rl_repo")
import numpy as np
import ml_dtypes
BF16 = ml_dtypes.bfloat16
B, C, H, W = 8, 256, 96, 96
HP, WP = H + 4, W + 4
SLABS = ((0, 8), (8, 16), (24, 16), (40, 16), (56, 16), (72, 16), (88, 8))
CHUNK = 4
OFFS = (-2, 0, 2)
MS = (0, 1, 2, 4, 5, 8)
_cache = {}

def _build(repeats=1, loop=None):
    import concourse.mybir as mybir
    from concourse import bacc
    from concourse.tile import TileContext
    dt = mybir.dt
    nc = bacc.Bacc("TRN2", target_bir_lowering=False, debug=False)
    xp = nc.dram_tensor("xp", (128, 2, HP, WP), dt.bfloat16, kind="ExternalInput").ap()
    cpl = nc.dram_tensor("cpl", (128, 6, H, W), dt.bfloat16, kind="ExternalInput").ap()
    wts = nc.dram_tensor("wts", (2, 128, 9 * 2 * 128), dt.bfloat16, kind="ExternalInput").ap()
    bias = nc.dram_tensor("bias", (128, 2), dt.float32, kind="ExternalInput").ap()
    ident = nc.dram_tensor("ident", (128, 128), dt.bfloat16, kind="ExternalInput").ap()
    y = nc.dram_tensor("y", (2, 128, H, W), dt.float32, kind="ExternalOutput").ap()
    with TileContext(nc) as tc:
        with (
            tc.tile_pool(name="const", bufs=1) as constp,
            tc.tile_pool(name="smpool", bufs=1) as smpool,
            tc.tile_pool(name="io", bufs=2) as iop,
            tc.tile_pool(name="tmp", bufs=1) as tmp,
            tc.tile_pool(name="outp", bufs=6) as outp,
            tc.tile_pool(name="psum", bufs=8, space="PSUM") as psp,
        ):
            id_sb = constp.tile([128, 128], dt.bfloat16)
            nc.sync.dma_start(out=id_sb, in_=ident)
            w_sb = constp.tile([128, 2, 9 * 2 * 128], dt.bfloat16)
            b_sb = constp.tile([128, 2], dt.float32)
            def load_consts():
                nc.sync.dma_start(out=w_sb[:, 0], in_=wts[0])
                nc.sync.dma_start(out=w_sb[:, 1], in_=wts[1])
                nc.sync.dma_start(out=b_sb, in_=bias)
            sm = smpool.tile([128, 2, HP, WP], dt.bfloat16)
            nc.vector.memset(sm[:, :, 0:2, :], 0.0)
            nc.vector.memset(sm[:, :, HP - 2 : HP, :], 0.0)
            nc.vector.memset(sm[:, :, 2 : HP - 2, 0:2], 0.0)
            nc.vector.memset(sm[:, :, 2 : HP - 2, WP - 2 : WP], 0.0)
            NR = 16
            def smooth_front(r0, nr):
                xs = iop.tile([128, 2, NR + 4, WP], dt.bfloat16, name="xs")[:, :, : nr + 4, :]
                nc.sync.dma_start(out=xs, in_=xp[:, :, r0 : r0 + nr + 4, :])
                cp = iop.tile([128, 6, NR, W], dt.bfloat16, name="cp")[:, :, :nr, :]
                ch = nr // 2 if nr > 8 else nr
                nc.sync.dma_start(out=cp[:, :, :ch], in_=cpl[:, :, r0 : r0 + ch, :])
                if ch < nr:
                    nc.sync.dma_start(out=cp[:, :, ch:nr], in_=cpl[:, :, r0 + ch : r0 + nr, :])
                def cpm(m):
                    i = MS.index(m)
                    return cp[:, i : i + 1].to_broadcast([128, 2, nr, W])
                P0 = xs[:, :, :, 2 : W + 2]
                P1 = tmp.tile([128, 2, NR + 4, W], dt.bfloat16, name="P1", bufs=2)[:, :, : nr + 4]
                nc.vector.tensor_add(P1, xs[:, :, :, 1 : W + 1], xs[:, :, :, 3 : W + 3])
                P2 = tmp.tile([128, 2, NR + 4, W], dt.bfloat16, name="P2", bufs=2)[:, :, : nr + 4]
                nc.vector.tensor_add(P2, xs[:, :, :, 0:W], xs[:, :, :, 4 : W + 4])
                ctr = lambda P: P[:, :, 2 : nr + 2]
                u1 = lambda P: P[:, :, 1 : nr + 1]
                d1 = lambda P: P[:, :, 3 : nr + 3]
                u2 = lambda P: P[:, :, 0:nr]
                d2 = lambda P: P[:, :, 4 : nr + 4]
                S5 = tmp.tile([128, 2, NR, W], dt.bfloat16, name="S5", bufs=2)[:, :, :nr]
                for ct in range(2):
                    for rk in range(nr // CHUNK):
                        rs = CHUNK * rk
                        pc5 = psp.tile([128, CHUNK, W], dt.float32, name="pc5", bufs=2)
                        for j, Pv in enumerate((u2(P1), d2(P1), u1(P2), d1(P2))):
                            nc.tensor.matmul(pc5, id_sb, Pv[:, ct, rs : rs + CHUNK, :],
                                             start=(j == 0), stop=(j == 3))
                        nc.scalar.activation(S5[:, ct, rs : rs + CHUNK, :], pc5,
                                             mybir.ActivationFunctionType.Copy)
                return xs, cp, P1, P2, S5

            def smooth_rest(r0, nr, xs, cp, P1, P2, S5, flush_fn=None, eng=None):
                eng = eng if eng is not None else nc.gpsimd
                P0 = xs[:, :, :, 2 : W + 2]
                S1 = tmp.tile([128, 2, NR, W], dt.bfloat16, name="S1")[:, :, :nr]
                S2 = tmp.tile([128, 2, NR, W], dt.bfloat16, name="S2")[:, :, :nr]
                S4 = tmp.tile([128, 2, NR, W], dt.bfloat16, name="S4")[:, :, :nr]
                S8 = tmp.tile([128, 2, NR, W], dt.bfloat16, name="S8")[:, :, :nr]
                halves = ((0, nr),) if nr <= 8 else ((0, nr // 2), (nr // 2, nr // 2))
                sl = lambda: None  # namespace for full-slab shifted views
                fctr = lambda P: P[:, :, 2 : 2 + nr]
                fu1 = lambda P: P[:, :, 1 : 1 + nr]
                fd1 = lambda P: P[:, :, 3 : 3 + nr]
                nc.vector.tensor_add(S1, fu1(P0), fd1(P0))
                nc.vector.tensor_add(S1, S1, fctr(P1))
                for h0, hn in halves:
                    hs = slice(h0, h0 + hn)
                    # row-shifted views of P (offset +2 = centered) restricted to this half
                    ctr = lambda P: P[:, :, 2 + h0 : 2 + h0 + hn]
                    u1 = lambda P: P[:, :, 1 + h0 : 1 + h0 + hn]
                    d1 = lambda P: P[:, :, 3 + h0 : 3 + h0 + hn]
                    u2 = lambda P: P[:, :, h0 : h0 + hn]
                    d2 = lambda P: P[:, :, 4 + h0 : 4 + h0 + hn]
                    eng.tensor_add(S2[:, :, hs], u1(P1), d1(P1))
                    eng.tensor_add(S4[:, :, hs], u2(P0), d2(P0))
                    eng.tensor_add(S4[:, :, hs], S4[:, :, hs], ctr(P2))
                    eng.tensor_add(S8[:, :, hs], u2(P2), d2(P2))
                    def cpmh(m):
                        i = MS.index(m)
                        return cp[:, i : i + 1, h0 : h0 + hn, :].to_broadcast([128, 2, hn, W])
                    acc = tmp.tile([128, 2, NR, W], dt.bfloat16, name="acc", bufs=2)[:, :, :hn]
                    nc.vector.tensor_mul(acc, ctr(P0), cpmh(0))
                    sm_out = sm[:, :, 2 + r0 + h0 : 2 + r0 + h0 + hn, 2 : W + 2]
                    def term(S, m, last=False):
                        t = tmp.tile([128, 2, NR, W], dt.bfloat16, name="t", bufs=2)[:, :, :hn]
                        nc.vector.tensor_mul(t, S[:, :, hs], cpmh(m))
                        nc.vector.tensor_add(sm_out if last else acc, acc, t)
                    term(S2, 2)
                    term(S4, 4)
                    term(S8, 8)
                    term(S1, 1)
                    term(S5, 5, last=True)
                    if flush_fn is not None:
                        flush_fn(r0 + h0 + hn)
            def conv_group(rrs):
                for oi in range(2):
                    pcs = [psp.tile([128, CHUNK, W], dt.float32, name="pc", bufs=6) for _ in rrs]
                    for idx in range(18):
                        ki, q = idx // 9, idx % 9
                        dh, dw = OFFS[q // 3], OFFS[q % 3]
                        lhsT = w_sb[:, ki, (q * 2 + oi) * 128 : (q * 2 + oi + 1) * 128]
                        for j, rr in enumerate(rrs):
                            rhs = sm[:, ki, 2 + rr + dh : 2 + rr + CHUNK + dh, 2 + dw : 2 + dw + W]
                            nc.tensor.matmul(pcs[j], lhsT, rhs, start=(idx == 0), stop=(idx == 17))
                    for j, rr in enumerate(rrs):
                        ob = outp.tile([128, CHUNK, W], dt.float32, name="ob")
                        nc.scalar.activation(ob, pcs[j], mybir.ActivationFunctionType.Relu,
                                             bias=b_sb[:, oi : oi + 1], scale=1.0)
                        nc.sync.dma_start(out=y[oi, :, rr : rr + CHUNK, :], in_=ob)
            def body():
                pending = list(range(0, H, CHUNK))
                def flush(upto):
                    ready = [rr for rr in pending if rr + 6 <= upto or upto >= H]
                    for rr in ready:
                        pending.remove(rr)
                    if ready:
                        conv_group(ready)
                prev_end = None
                last = len(SLABS) - 1
                for si, (r0, nr) in enumerate(SLABS):
                    fr = smooth_front(r0, nr)
                    if si == 0:
                        load_consts()
                    if prev_end is not None:
                        flush(prev_end)
                    smooth_rest(r0, nr, *fr,
                                flush_fn=flush,
                                eng=nc.vector)
                    prev_end = r0 + nr
                flush(H)
                assert not pending
            if loop is not None:
                with tc.For_i(0, loop, 1):
                    body()
            else:
                for _ in range(repeats):
                    body()
    nc.compile()
    return nc

def _prep(inputs):
    x = np.asarray(inputs["x"], np.float32)
    pm = np.asarray(inputs["perspective_map"], np.float32)
    co = np.asarray(inputs["sigma_coeffs"], np.float32)
    Wc = np.asarray(inputs["conv_w"], np.float32)
    bb = np.asarray(inputs["conv_b"], np.float32)
    p = pm[:, 0]
    sigma = np.maximum(co[0] * p**3 + co[1] * p**2 + co[2] * p + co[3], 0.5)
    t = np.exp(-1.0 / (2.0 * sigma * sigma))
    Z = 1 + 4 * t + 4 * t**2 + 4 * t**4 + 8 * t**5 + 4 * t**8
    cm = np.stack([(t**m) / Z for m in MS], axis=1).astype(BF16)
    cpl = np.ascontiguousarray(np.broadcast_to(cm[:, None], (B, 128, 6, H, W)))
    xpad = np.zeros((B, 128, 2, HP, WP), BF16)
    xpad[:, :, :, 2 : H + 2, 2 : W + 2] = (
        x.astype(BF16).reshape(B, 2, 128, H, W).transpose(0, 2, 1, 3, 4))
    Wt = Wc.transpose(1, 0, 2, 3).astype(BF16)
    wts = np.empty((2, 128, 9, 2, 128), BF16)
    for ki in range(2):
        for q in range(9):
            kh, kw = q // 3, q % 3
            for oi in range(2):
                wts[ki, :, q, oi, :] = Wt[ki * 128 : (ki + 1) * 128, oi * 128 : (oi + 1) * 128, kh, kw]
    wts = wts.reshape(2, 128, 9 * 2 * 128)
    bias_h = np.ascontiguousarray(bb.reshape(2, 128).T.astype(np.float32))
    ident = np.eye(128, dtype=BF16)
    return [{"xp": xpad[b], "cpl": cpl[b], "wts": wts, "bias": bias_h, "ident": ident} for b in range(B)]

def _get_nc(repeats=1, loop=None, **kw):
    key = ("nc", repeats, loop)
    if key not in _cache:
        _cache[key] = _build(repeats, loop)
    return _cache[key]


def run(inputs, trace=False, **kw):
    from concourse.bass_utils import run_bass_kernel_spmd

    nc = _get_nc()
    in_maps = _prep(inputs)
    res = run_bass_kernel_spmd(nc, in_maps, core_ids=list(range(B)), trace=trace, **kw)
    out = np.stack([r["y"].reshape(C, H, W) for r in res.results]).astype(np.float32)
    return out, res


def kernel(**inputs):
    out, _ = run(inputs)
    return out

